# revision 18
# baseline (speedup 1.0000x reference)
"""AttentiveFP forward pass as a Bass/Tile kernel on 8 Trainium2 NeuronCores.

v2: bf16 matmuls (4x PE rate + FWL weight loads), host-precomputed edge
selection matrices, aggregation of the gathered hl rows directly (no hrT
reconstruction), GRU batched over 512-node column groups with stationary
weights, Exp-only ACT function in the chunk sweep (Lrelu/Relu/Elu built from
DVE min/max + exp), column-wise reciprocal for the softmax normalizer, and
double-buffered node tables with quarter-chunked AllGathers overlapping
compute.

Data-parallel by graph blocks (256 graphs/core); edges assigned to the core
owning their dst node; per-core windowed segment-softmax aggregation via
selection-matrix matmuls on the PE; per-edge source rows fetched with
indirect DMA gathers from the AllGathered table.
"""
import sys, os
sys.path.insert(0, '/opt/trn_rl_repo')
import numpy as np
import ml_dtypes
from contextlib import ExitStack

import concourse.bass as bass
import concourse.mybir as mybir
import concourse.tile as tile
from concourse.bass import IndirectOffsetOnAxis
from concourse.mybir import AluOpType as alu, ActivationFunctionType as act

BF16 = ml_dtypes.bfloat16
BF = mybir.dt.bfloat16
F32 = mybir.dt.float32
I32 = mybir.dt.int32
EPS = 1e-30

# ---------------- walrus sync-wait splitting ----------------
MAX_WAITS = 1

def split_waits(nc):
    eng_map = nc.engines
    for bbname, bassbb in nc.bb_map.items():
        insts = bassbb.bb.instructions
        i = 0
        while i < len(insts):
            inst = insts[i]
            si = inst.sync_info
            if si is not None and si.on_wait is not None and len(si.on_wait) > MAX_WAITS:
                waits = list(si.on_wait)
                si.on_wait = waits[-MAX_WAITS:]
                rest = waits[:-MAX_WAITS]
                for j in range(0, len(rest), MAX_WAITS):
                    eng = eng_map[inst.engine]
                    nop = eng.nop(nofuse=True)
                    nop_inst = nop.ins
                    for obb in nc.bb_map.values():
                        lst = obb.bb.instructions
                        for k in range(len(lst) - 1, -1, -1):
                            if lst[k].name == nop_inst.name:
                                del lst[k]
                                break
                    nsi = nop_inst.sync_info
                    chunk = rest[j:j + MAX_WAITS]
                    if nsi is None:
                        nop_inst.sync_info = mybir.SyncInfo(on_wait=chunk, on_update=[])
                    else:
                        nsi.on_wait = chunk
                    insts.insert(i, nop_inst)
                    i += 1
            i += 1


class TileContextFixed(tile.TileContext):
    def __exit__(self, *args):
        r = super().__exit__(*args)
        split_waits(self.nc)
        return r


def preprocess(edge_index, batch, n_cores=8, G=2048, CW=5):
    src = np.asarray(edge_index[0]).astype(np.int64)
    dst = np.asarray(edge_index[1]).astype(np.int64)
    batch = np.asarray(batch).astype(np.int64)
    N = batch.shape[0]
    GPC = G // n_cores
    gstart = np.searchsorted(batch, np.arange(0, G + 1, GPC))
    ncounts = np.diff(gstart)
    NLOC = int(np.ceil(ncounts.max() / 512) * 512)
    NWIN = NLOC // 128
    NLQ = NLOC // 4

    node_owner = np.searchsorted(gstart, np.arange(N), side='right') - 1
    loc = np.arange(N) - gstart[node_owner]
    q = loc // NLQ
    ag_row = (q * (n_cores * NLQ) + node_owner * NLQ + (loc % NLQ)).astype(np.int64)
    owner = node_owner[dst]
    ar128 = np.arange(128)

    cores = []
    for c in range(n_cores):
        ns, ne = int(gstart[c]), int(gstart[c + 1])
        nn = ne - ns
        m = owner == c
        eidx = np.nonzero(m)[0]
        dl = dst[eidx] - ns
        order = np.argsort(dl, kind='stable')
        eidx = eidx[order]; dl = dl[order]
        win = dl // 128
        counts = np.bincount(win, minlength=NWIN)
        assert counts.max() <= CW * 128, f"window overflow {counts.max()}"
        pos = np.concatenate([[0], np.cumsum(counts)])[:-1]
        within = np.arange(len(dl)) - pos[win]
        slots = (win * CW * 128 + within).astype(np.int64)

        sl_src = np.zeros(NWIN * CW * 128, np.int32)
        sl_dl = np.full(NWIN * CW * 128, -1.0, np.float32)
        sl_edge = np.zeros(NWIN * CW * 128, np.int64)
        sl_fill = np.zeros(NWIN * CW * 128, bool)
        sl_src[slots] = ag_row[src[eidx]]
        sl_dl[slots] = (dl % 128).astype(np.float32)
        sl_edge[slots] = eidx
        sl_fill[slots] = True

        srcblk = sl_src.reshape(NWIN, CW, 128).transpose(0, 2, 1).copy()
        dlblk = sl_dl.reshape(NWIN, CW, 128).transpose(0, 2, 1).copy()
        # selT [w, d, c*128+p] = 1 if dstloc(w,p,c)==d
        selw = (dlblk.transpose(0, 2, 1)[:, :, None, :] ==
                ar128[None, None, :, None])                  # [w, c, d, p]
        selTblk = selw.transpose(0, 2, 1, 3).reshape(
            NWIN, 128, CW * 128).astype(BF16)

        gloc = np.full(NLOC, -1.0, np.float32)
        gloc[:nn] = (batch[ns:ne] - c * GPC).astype(np.float32)
        glocT = gloc.reshape(NWIN, 128).T.copy()             # [128, NWIN]
        gw = gloc.reshape(NWIN, 128)
        selgT = np.zeros((NWIN, 128, 256), BF16)
        for gb in range(2):
            selgT[:, :, gb * 128:(gb + 1) * 128] = (
                gw[:, None, :] == (gb * 128 + ar128)[None, :, None])

        cores.append(dict(ns=ns, ne=ne, nn=nn,
                          srcblk=srcblk, dlblk=dlblk, selTblk=selTblk,
                          glocT=glocT, selgT=selgT,
                          sl_edge=sl_edge, sl_fill=sl_fill))
    return dict(cores=cores, gstart=gstart, NLOC=NLOC, NWIN=NWIN, NLQ=NLQ,
                CW=CW, GPC=GPC, n_cores=n_cores)


def wpb_layout():
    """bf16 pack: name -> (off, cols)."""
    L = {}
    off = 0
    def add(name, cols):
        nonlocal off
        L[name] = (off, cols)
        off += cols
    add("iota_sq", 128)
    add("iota256", 256)
    add("ones_col", 1)
    add("ident", 128)
    add("attg", 256)
    for l in range(3):
        add(f"atta{l}", 256)
    add("attm", 256)
    for b in range(2):
        add(f"W1A{b}", 257)
    for i in range(4):
        add(f"WlTr{i}", 512)
        add(f"WrTr{i}", 512)
    add("id0", 256)
    add("id1", 256)
    for k in range(2):
        for b in range(2):
            add(f"gl2T_{k}{b}", 128)
    for g in range(5):
        for j in range(12):
            for b in range(2):
                add(f"gru{g}_w{j}{b}", 128)
    add("w1T", 256)
    add("w2T", 64)
    return L, off


def wpf_layout():
    """f32 pack (biases): name -> (off, cols)."""
    L = {}
    off = 0
    def add(name, cols):
        nonlocal off
        L[name] = (off, cols)
        off += cols
    for g in range(5):
        for j in range(4):
            for b in range(2):
                add(f"gru{g}_b{j}{b}", 1)
    for l in range(3):
        for b in range(2):
            add(f"ab{l}{b}", 1)
    for b in range(2):
        add(f"gb2{b}", 1)
    for b in range(2):
        add(f"molb{b}", 1)
    add("b1", 1)
    add("b2_", 1)
    return L, off


def make_wpacks(inp):
    Lb, WB = wpb_layout()
    Lf, WF = wpf_layout()
    Wb = np.zeros((128, WB), np.float32)
    Wf = np.zeros((128, WF), np.float32)

    def putb(name, arr):
        off, cols = Lb[name]
        assert arr.shape == (128, cols), (name, arr.shape, cols)
        Wb[:, off:off + cols] = arr

    def putf(name, arr):
        off, cols = Lf[name]
        assert arr.shape == (128, cols), (name, arr.shape, cols)
        Wf[:, off:off + cols] = arr

    putb("iota_sq", np.tile(np.arange(128, dtype=np.float32), (128, 1)))
    putb("iota256", np.tile(np.arange(256, dtype=np.float32), (128, 1)))
    putb("ones_col", np.ones((128, 1), np.float32))
    I = np.eye(128, dtype=np.float32)
    putb("ident", I)
    putb("attg", np.tile(np.asarray(inp['g_att_l'], np.float32), (128, 1)))
    for l in range(3):
        putb(f"atta{l}", np.tile(np.asarray(inp['atom_att'][l], np.float32), (128, 1)))
    putb("attm", np.tile(np.asarray(inp['mol_att'], np.float32), (128, 1)))
    W1T = np.asarray(inp['g_lin1_w'], np.float32)[:, :256].T     # [256 in, 256 out]
    attr = np.asarray(inp['g_att_r'], np.float32).reshape(2, 128).T  # [128, 2]
    for b in range(2):
        putb(f"W1A{b}", np.concatenate(
            [W1T[b * 128:(b + 1) * 128], attr[:, b:b + 1]], axis=1))
    Wls = [np.asarray(inp['atom_Wl'][0]), np.asarray(inp['atom_Wl'][1]),
           np.asarray(inp['atom_Wl'][2]), np.asarray(inp['mol_Wl'])]
    Wrs = [np.asarray(inp['atom_Wr'][0]), np.asarray(inp['atom_Wr'][1]),
           np.asarray(inp['atom_Wr'][2]), np.asarray(inp['mol_Wr'])]
    for i in range(4):
        WT = Wls[i].T.astype(np.float32)                         # [256 k, 256 out]
        putb(f"WlTr{i}", np.concatenate([WT[0:128], WT[128:256]], axis=1))
        WT = Wrs[i].T.astype(np.float32)
        putb(f"WrTr{i}", np.concatenate([WT[0:128], WT[128:256]], axis=1))
    putb("id0", np.concatenate([I, np.zeros((128, 128), np.float32)], 1))
    putb("id1", np.concatenate([np.zeros((128, 128), np.float32), I], 1))
    g2T = np.asarray(inp['g_lin2_w'], np.float32).T              # [k, out]
    for k in range(2):
        for b in range(2):
            putb(f"gl2T_{k}{b}", g2T[k * 128:(k + 1) * 128, b * 128:(b + 1) * 128])
    grus = [('gru0_wih', 'gru0_whh', 'gru0_bih', 'gru0_bhh', None),
            ('agru_wih', 'agru_whh', 'agru_bih', 'agru_bhh', 0),
            ('agru_wih', 'agru_whh', 'agru_bih', 'agru_bhh', 1),
            ('agru_wih', 'agru_whh', 'agru_bih', 'agru_bhh', 2),
            ('mgru_wih', 'mgru_whh', 'mgru_bih', 'mgru_bhh', None)]
    for g, (wi, wh, bi, bh, l) in enumerate(grus):
        wih = np.asarray(inp[wi] if l is None else inp[wi][l], np.float32)
        whh = np.asarray(inp[wh] if l is None else inp[wh][l], np.float32)
        bih = np.asarray(inp[bi] if l is None else inp[bi][l], np.float32)
        bhh = np.asarray(inp[bh] if l is None else inp[bh][l], np.float32)
        wihT = wih.T    # [256 k, 768]
        whhT = whh.T
        # j: 0,1 wih-r (input side); 2,3 whh-r (hidden side); 4..7 z; 8,9 wih-n; 10,11 whh-n
        for k in range(2):
            for b in range(2):
                ks, bs = slice(k * 128, (k + 1) * 128), slice(b * 128, (b + 1) * 128)
                putb(f"gru{g}_w{0 + k}{b}", wihT[ks, 0:256][:, bs])
                putb(f"gru{g}_w{2 + k}{b}", whhT[ks, 0:256][:, bs])
                putb(f"gru{g}_w{4 + k}{b}", wihT[ks, 256:512][:, bs])
                putb(f"gru{g}_w{6 + k}{b}", whhT[ks, 256:512][:, bs])
                putb(f"gru{g}_w{8 + k}{b}", wihT[ks, 512:768][:, bs])
                putb(f"gru{g}_w{10 + k}{b}", whhT[ks, 512:768][:, bs])
        br = (bih[0:256] + bhh[0:256]).reshape(2, 128).T
        bz = (bih[256:512] + bhh[256:512]).reshape(2, 128).T
        bin_ = bih[512:768].reshape(2, 128).T
        bhn = bhh[512:768].reshape(2, 128).T
        for j, arr in enumerate([br, bz, bin_, bhn]):
            for b in range(2):
                putf(f"gru{g}_b{j}{b}", arr[:, b:b + 1])
    ab = np.asarray(inp['atom_bias'], np.float32)
    for l in range(3):
        for b in range(2):
            putf(f"ab{l}{b}", ab[l].reshape(2, 128).T[:, b:b + 1])
    gb2 = np.asarray(inp['g_bias'], np.float32).reshape(2, 128).T
    molb = np.asarray(inp['mol_bias'], np.float32).reshape(2, 128).T
    for b in range(2):
        putf(f"gb2{b}", gb2[:, b:b + 1])
        putf(f"molb{b}", molb[:, b:b + 1])
    putf("b1", np.asarray(inp['mlp_b1'], np.float32).reshape(128, 1))
    putf("b2_", np.pad(np.asarray(inp['mlp_b2'], np.float32), (0, 64)).reshape(128, 1))
    w1T = np.asarray(inp['mlp_w1'], np.float32).T                # [256, 128]
    putb("w1T", np.concatenate([w1T[0:128], w1T[128:256]], 1))
    putb("w2T", np.asarray(inp['mlp_w2'], np.float32).T)         # [128, 64]
    return Wb.astype(BF16), Wf


def build_kernel(NLOC, NWIN, CW, NG, n_cores):
    H = 256
    NLQ = NLOC // 4
    NGRP = NWIN // 4
    NWQ = NWIN // 4            # windows per AG quarter

    nc = bass.Bass(num_devices=n_cores)
    Lb, WB = wpb_layout()
    Lf, WF = wpf_layout()

    def dram_in(name, shape, dt=BF):
        return nc.dram_tensor(name, list(shape), dt, kind="ExternalInput")

    xinT = dram_in("xinT", [65, NLOC])
    srcblk = dram_in("srcblk", [NWIN, 128, CW], I32)
    dlblk = dram_in("dlblk", [NWIN, 128, CW], F32)
    selTblk = dram_in("selTblk", [NWIN, 128, CW * 128])
    eaT2 = dram_in("eaT2", [NWIN, 16, CW * 128])
    glocTd = dram_in("glocT", [128, NWIN], F32)
    selgTblk = dram_in("selgTblk", [NWIN, 128, 256])
    wpbd = dram_in("wpb", [128, WB])
    wpfd = dram_in("wpf", [128, WF], F32)
    lin1Td = dram_in("lin1T", [65, 256])
    W2Td = dram_in("W2T", [16, 256])
    w3Td = dram_in("w3T", [65, 1])

    y = nc.dram_tensor("y", [1, 256], F32, kind="ExternalOutput")

    cc_in = nc.dram_tensor("cc_in", [NLOC, H], BF)
    tabs = [nc.dram_tensor(f"tab{i}", [n_cores * NLOC, H], BF, addr_space="Shared")
            for i in range(2)]
    xld = nc.dram_tensor("xld", [NLOC, H], BF)

    with TileContextFixed(nc) as tc, ExitStack() as ctx:
        wpool = ctx.enter_context(tc.tile_pool(name="weights", bufs=1))
        mpool = ctx.enter_context(tc.tile_pool(name="meta", bufs=2))
        gpool = ctx.enter_context(tc.tile_pool(name="gath", bufs=6))
        vpool = ctx.enter_context(tc.tile_pool(name="vals", bufs=6))
        npool = ctx.enter_context(tc.tile_pool(name="node", bufs=3))
        spool = ctx.enter_context(tc.tile_pool(name="small", bufs=4))
        upool = ctx.enter_context(tc.tile_pool(name="gru", bufs=2))
        pp = ctx.enter_context(tc.tile_pool(name="ps", bufs=1, space="PSUM"))

        wpb = wpool.tile([128, WB], BF, tag="wpb")
        nc.sync.dma_start(wpb[:], wpbd.ap())
        wpf = wpool.tile([128, WF], F32, tag="wpf")
        nc.sync.dma_start(wpf[:], wpfd.ap())

        def W(name):
            off, cols = Lb[name]
            return wpb[:, off:off + cols]

        def F(name):
            off, cols = Lf[name]
            return wpf[:, off:off + cols]

        iota_sq = W("iota_sq")
        ident = W("ident")
        ones_col = W("ones_col")
        lin1T = wpool.tile([65, 256], BF, tag="lin1T")
        nc.sync.dma_start(lin1T[:], lin1Td.ap())
        W2T = wpool.tile([16, 256], BF, tag="W2T")
        nc.sync.dma_start(W2T[:], W2Td.ap())
        w3T = wpool.tile([65, 1], BF, tag="w3T")
        nc.sync.dma_start(w3T[:], w3Td.ap())
        ones1 = wpool.tile([1, 128], BF, tag="ones1")
        nc.vector.memset(ones1[:], 1.0)
        onesq = wpool.tile([1, 1], BF, tag="onesq")
        nc.vector.memset(onesq[:], 1.0)
        ones1f = wpool.tile([1, 128], F32, tag="ones1f")
        nc.vector.memset(ones1f[:], 1.0)
        onesqf = wpool.tile([1, 1], F32, tag="onesqf")
        nc.vector.memset(onesqf[:], 1.0)
        identff = wpool.tile([128, 128], F32, tag="identff")
        nc.vector.tensor_copy(identff[:], ident)  # bf16 identity -> f32
        glocT = wpool.tile([128, NWIN], F32, tag="glocT")
        nc.sync.dma_start(glocT[:], glocTd.ap())

        # persistent node-state tiles
        xts = [wpool.tile([128, NLOC], BF, tag=f"xT{b}", name=f"xT{b}") for b in range(2)]
        hrR = wpool.tile([128, NWIN * 256], BF, tag="hrR")
        wc = wpool.tile([128, NWIN], BF, tag="wc")
        outTs = [wpool.tile([128, NG], BF, tag=f"outT{b}", name=f"outT{b}") for b in range(2)]

        def ps_tile(tag, name):
            return pp.tile([128, 512], F32, tag=tag, name=name, bufs={
                "agg": 2, "ch": 2, "gp": 3}[tag])

        def launch_ag(dst_tab, q):
            rs = cc_in.ap()[q * NLQ:(q + 1) * NLQ, :]
            os_ = dst_tab.ap()[q * n_cores * NLQ:(q + 1) * n_cores * NLQ, :]
            nc.gpsimd.collective_compute(
                "AllGather", alu.bypass,
                replica_groups=[list(range(n_cores))],
                ins=[rs], outs=[os_])

        def ag_quarters(grp):
            return [q for q in range(4) if ((q + 1) * NWQ - 1) // 4 == grp]

        # ---------- softmax normalizer: row sums -> rbc [128,128] ----------
        def make_rbc(agg, sums_cols):
            """agg bank holds row [1,128] of exp-sums at sums_cols; returns
            rbc sbuf tile [128,128] f32 with 1/sum broadcast down columns."""
            sums_sb = spool.tile([1, 128], F32, tag="sums_sb", name="sums_sb")
            nc.vector.tensor_scalar(out=sums_sb[:], in0=agg[0:1, sums_cols],
                                    scalar1=EPS, scalar2=None, op0=alu.max)
            cps = ps_tile("ch", "colps")
            nc.tensor.matmul(cps[:, 0:1], lhsT=sums_sb[:], rhs=onesqf[:],
                             start=True, stop=True)
            recipc = spool.tile([128, 1], F32, tag="recipc", name="recipc")
            nc.vector.reciprocal(recipc[:], cps[:, 0:1])
            rps = ps_tile("ch", "rowps")
            nc.tensor.matmul(rps[0:1, 0:128], lhsT=recipc[:], rhs=identff[:],
                             start=True, stop=True)
            rrow = spool.tile([1, 128], F32, tag="rrow", name="rrow")
            nc.vector.tensor_copy(rrow[:], rps[0:1, 0:128])
            bps = ps_tile("ch", "bcps")
            nc.tensor.matmul(bps[:, 0:128], lhsT=ones1f[:], rhs=rrow[:],
                             start=True, stop=True)
            rbc = npool.tile([128, 128], F32, tag="rbc", name="rbc")
            nc.vector.tensor_copy(rbc[:], bps[:, 0:128])
            return rbc

        def elu_into(dst_ap, t, bias_ap, fd):
            """dst = elu(t + bias); t is sbuf bf16 [128, fd]."""
            m = npool.tile([128, fd], F32, tag=f"elu_m{fd}", name="elum")
            nc.vector.tensor_scalar(out=m[:], in0=t[:], scalar1=bias_ap,
                                    scalar2=0.0, op0=alu.add, op1=alu.min)
            r = npool.tile([128, fd], F32, tag=f"elu_r{fd}", name="elur")
            nc.vector.tensor_scalar(out=r[:], in0=t[:], scalar1=bias_ap,
                                    scalar2=0.0, op0=alu.add, op1=alu.max)
            e = npool.tile([128, fd], F32, tag=f"elu_e{fd}", name="elue")
            nc.scalar.activation(e[:], m[:], act.Exp)
            nc.vector.scalar_tensor_tensor(out=dst_ap, in0=e[:], scalar=-1.0,
                                           in1=r[:], op0=alu.add, op1=alu.add)

        # ---------------- batched GRU ----------------
        def gru_batched(g, hTg, kspan, xsl, ncols):
            """hTg: [128, 2*kspan] bf16 (input feature half k at cols k*kspan);
            hidden state = xts-like tiles given by closure target `gxts`;
            writes relu'd new state back into gxts[b][:, xsl]."""
            gxts = xts if g < 4 else outTs

            def gate_ps(b, jh, jx, name):
                p = ps_tile("gp", name)
                nmm = (2 if jh is not None else 0) + (2 if jx is not None else 0)
                i = 0
                for k in range(2):
                    if jh is not None:
                        nc.tensor.matmul(p[:, 0:ncols],
                                         lhsT=W(f"gru{g}_w{jh + k}{b}"),
                                         rhs=hTg[:, k * kspan:k * kspan + ncols],
                                         start=(i == 0), stop=(i == nmm - 1))
                        i += 1
                for k in range(2):
                    if jx is not None:
                        nc.tensor.matmul(p[:, 0:ncols],
                                         lhsT=W(f"gru{g}_w{jx + k}{b}"),
                                         rhs=gxts[k][:, xsl],
                                         start=(i == 0), stop=(i == nmm - 1))
                        i += 1
                return p

            rps = [gate_ps(b, 0, 2, "rps") for b in range(2)]
            r = []
            for b in range(2):
                t = upool.tile([128, ncols], F32, tag="gru_r", name="grur", bufs=1)
                nc.scalar.activation(t[:], rps[b][:, 0:ncols], act.Sigmoid,
                                     bias=F(f"gru{g}_b0{b}"))
                r.append(t)
            zps = [gate_ps(b, 4, 6, "zps") for b in range(2)]
            z = []
            for b in range(2):
                t = upool.tile([128, ncols], F32, tag="gru_z", name="gruz", bufs=1)
                nc.scalar.activation(t[:], zps[b][:, 0:ncols], act.Sigmoid,
                                     bias=F(f"gru{g}_b1{b}"))
                z.append(t)
            hps = [gate_ps(b, None, 10, "hps") for b in range(2)]
            t1 = []
            for b in range(2):
                t = upool.tile([128, ncols], F32, tag="gru_t1", name="grut1", bufs=2)
                nc.vector.scalar_tensor_tensor(out=t[:], in0=hps[b][:, 0:ncols],
                                               scalar=F(f"gru{g}_b3{b}"),
                                               in1=r[b][:], op0=alu.add,
                                               op1=alu.mult)
                t1.append(t)
            ips = [gate_ps(b, 8, None, "ips") for b in range(2)]
            t2 = []
            for b in range(2):
                t = upool.tile([128, ncols], F32, tag="gru_t2", name="grut2", bufs=2)
                nc.vector.tensor_tensor(out=t[:], in0=ips[b][:, 0:ncols],
                                        in1=t1[b][:], op=alu.add)
                t2.append(t)
            n = []
            for b in range(2):
                t = upool.tile([128, ncols], F32, tag="gru_n", name="grun", bufs=1)
                nc.scalar.activation(t[:], t2[b][:], act.Tanh,
                                     bias=F(f"gru{g}_b2{b}"))
                n.append(t)
            for b in range(2):
                d = upool.tile([128, ncols], F32, tag="gru_t1", name="grud", bufs=2)
                nc.vector.tensor_tensor(out=d[:], in0=gxts[b][:, xsl],
                                        in1=n[b][:], op=alu.subtract)
                zd = upool.tile([128, ncols], F32, tag="gru_t2", name="gruzd", bufs=2)
                nc.vector.tensor_tensor(out=zd[:], in0=z[b][:], in1=d[:],
                                        op=alu.mult)
                s = upool.tile([128, ncols], F32, tag="gru_s", name="grus", bufs=1)
                nc.vector.tensor_tensor(out=s[:], in0=n[b][:], in1=zd[:],
                                        op=alu.add)
                nc.vector.tensor_scalar(out=gxts[b][:, xsl], in0=s[:],
                                        scalar1=0.0, scalar2=None, op0=alu.max)

        # ---------------- table production (per window) ----------------
        def table_rows(li, w, last):
            wsl = slice(w * 128, (w + 1) * 128)
            p = ps_tile("ch", "tabps")
            for k in range(2):
                nc.tensor.matmul(p[:, 0:256], lhsT=xts[k][:, wsl],
                                 rhs=W(f"WlTr{li}")[:, k * 256:(k + 1) * 256],
                                 start=(k == 0), stop=(k == 1))
            if last:
                # hl_m rows for the mol phase live in hrR
                nc.vector.tensor_copy(hrR[:, w * 256:(w + 1) * 256], p[:, 0:256])
                p2 = ps_tile("ch", "tabps2")
                for k in range(2):
                    nc.tensor.matmul(p2[:, 0:256], lhsT=xts[k][:, wsl],
                                     rhs=W("id0") if k == 0 else W("id1"),
                                     start=(k == 0), stop=(k == 1))
                xsb = npool.tile([128, 256], BF, tag="cp", name="xsb")
                nc.vector.tensor_copy(xsb[:], p2[:, 0:256])
                nc.sync.dma_start(xld.ap()[wsl, :], xsb[:])
            else:
                hsb = npool.tile([128, 256], BF, tag="cp", name="hsb")
                nc.vector.tensor_copy(hsb[:], p[:, 0:256])
                nc.sync.dma_start(cc_in.ap()[wsl, :], hsb[:])
                p2 = ps_tile("ch", "tabps2")
                for k in range(2):
                    nc.tensor.matmul(p2[:, 0:256], lhsT=xts[k][:, wsl],
                                     rhs=W(f"WrTr{li}")[:, k * 256:(k + 1) * 256],
                                     start=(k == 0), stop=(k == 1))
                nc.vector.tensor_copy(hrR[:, w * 256:(w + 1) * 256], p2[:, 0:256])

        # ================= P0: input projection + u table =================
        for grp in range(NGRP):
            gsl = slice(grp * 512, (grp + 1) * 512)
            xing = mpool.tile([65, 512], BF, tag="xing")
            nc.sync.dma_start(xing[:], xinT.ap()[:, gsl])
            for b in range(2):
                p = ps_tile("gp", "p0ps")
                nc.tensor.matmul(p[:, 0:512], lhsT=lin1T[:, b * 128:(b + 1) * 128],
                                 rhs=xing[:], start=True, stop=True)
                nc.scalar.activation(xts[b][:, gsl], p[:, 0:512], act.Lrelu,
                                     alpha=0.01)
            for wi in range(4):
                w = grp * 4 + wi
                wsl = slice(w * 128, (w + 1) * 128)
                p = ps_tile("ch", "ups")
                for b in range(2):
                    nc.tensor.matmul(p[:, 0:257], lhsT=xts[b][:, wsl],
                                     rhs=W(f"W1A{b}"), start=(b == 0),
                                     stop=(b == 1))
                usb = npool.tile([128, 256], BF, tag="cp", name="usb")
                nc.vector.tensor_copy(usb[:], p[:, 0:256])
                nc.sync.dma_start(cc_in.ap()[wsl, :], usb[:])
                nc.vector.tensor_copy(wc[:, w:w + 1], p[:, 256:257])
            for q in ag_quarters(grp):
                launch_ag(tabs[0], q)

        # ================= edge layers =================
        def edge_layer(li, kind, src_tab, has_ag):
            gate = kind == 'gate'
            attw = W("attg") if gate else W(f"atta{li - 1}")
            dst_tab = tabs[(li + 1) % 2]
            for grp in range(NGRP):
                hTg = upool.tile([128, 1024], BF, tag="hTg", name="hTg")
                for wi in range(4):
                    w = grp * 4 + wi
                    wsl = slice(w * 128, (w + 1) * 128)
                    srct = mpool.tile([128, CW], I32, tag="srct")
                    nc.sync.dma_start(srct[:], srcblk.ap()[w])
                    dlc = mpool.tile([128, CW], F32, tag="dlc")
                    nc.sync.dma_start(dlc[:], dlblk.ap()[w])
                    selTw = mpool.tile([128, CW * 128], BF, tag="selTw")
                    nc.sync.dma_start(selTw[:], selTblk.ap()[w])
                    if gate:
                        eatw = mpool.tile([16, CW * 128], BF, tag="eatw")
                        nc.sync.dma_start(eatw[:], eaT2.ap()[w])

                    agg = ps_tile("agg", "agg")
                    ec = spool.tile([128, CW], F32, tag="ec", name="ec")
                    vals = []
                    for ci in range(CW):
                        csl = slice(ci * 128, (ci + 1) * 128)
                        g = gpool.tile([128, H], BF, tag="g", name="g")
                        nc.gpsimd.indirect_dma_start(
                            out=g[:], out_offset=None, in_=src_tab.ap(),
                            in_offset=IndirectOffsetOnAxis(
                                ap=srct[:, ci:ci + 1], axis=0))
                        ch = ps_tile("ch", "chps")
                        if gate:
                            nc.tensor.matmul(ch[:, 0:256], lhsT=eatw[:, csl],
                                             rhs=W2T[:], start=True, stop=False)
                        else:
                            nc.tensor.matmul(ch[:, 0:256], lhsT=selTw[:, csl],
                                             rhs=hrR[:, w * 256:(w + 1) * 256],
                                             start=True, stop=False)
                        nc.tensor.matmul(ch[:, 0:256], lhsT=ident, rhs=g[:],
                                         start=False, stop=True)
                        if gate:
                            # (x @ att_r)[dst] column; first touch of agg bank
                            nc.tensor.matmul(agg[:, 384 + ci:385 + ci],
                                             lhsT=selTw[:, csl],
                                             rhs=wc[:, w:w + 1],
                                             start=(ci == 0), stop=False,
                                             skip_group_check=True)
                        tsb = npool.tile([128, H], BF, tag="tsb", name="tsb")
                        nc.vector.tensor_copy(tsb[:], ch[:, 0:256])
                        if gate:
                            tl = vpool.tile([128, H], BF, tag="val", name="tl")
                        else:
                            tl = npool.tile([128, H], BF, tag="tls", name="tl")
                        nc.vector.scalar_tensor_tensor(
                            out=tl[:], in0=tsb[:], scalar=0.01, in1=tsb[:],
                            op0=alu.mult, op1=alu.max)
                        escr = npool.tile([128, H], BF, tag="escr", name="escr")
                        nc.vector.scalar_tensor_tensor(
                            out=escr[:], in0=tl[:], scalar=1.0, in1=attw,
                            op0=alu.mult, op1=alu.mult,
                            accum_out=ec[:, ci:ci + 1])
                        vals.append(tl if gate else g)

                    if gate:
                        e2 = spool.tile([128, CW], F32, tag="e2", name="e2")
                        nc.vector.scalar_tensor_tensor(
                            out=e2[:], in0=ec[:], scalar=0.0,
                            in1=agg[:, 384:384 + CW], op0=alu.add, op1=alu.add)
                        el = spool.tile([128, CW], F32, tag="el", name="el")
                        nc.vector.scalar_tensor_tensor(
                            out=el[:], in0=e2[:], scalar=0.01, in1=e2[:],
                            op0=alu.mult, op1=alu.max)
                        ex = spool.tile([128, CW], F32, tag="ex", name="ex")
                        nc.scalar.activation(ex[:], el[:], act.Exp)
                    else:
                        ex = spool.tile([128, CW], F32, tag="ex", name="ex")
                        nc.scalar.activation(ex[:], ec[:], act.Exp)

                    for ci in range(CW):
                        selw = spool.tile([128, 128], BF, tag="selw", name="selw")
                        nc.vector.tensor_scalar(
                            out=selw[:], in0=iota_sq, scalar1=dlc[:, ci:ci + 1],
                            scalar2=ex[:, ci:ci + 1], op0=alu.is_equal,
                            op1=alu.mult)
                        first = (ci == 0) and not gate
                        for b in range(2):
                            nc.tensor.matmul(
                                agg[:, b * 128:(b + 1) * 128],
                                lhsT=vals[ci][:, b * 128:(b + 1) * 128],
                                rhs=selw[:], start=(first and b == 0),
                                stop=(ci == CW - 1), skip_group_check=True)
                        nc.tensor.matmul(agg[0:1, 256:384], lhsT=ones_col,
                                         rhs=selw[:], start=False,
                                         stop=(ci == CW - 1),
                                         skip_group_check=True)

                    # ---- epilogue ----
                    rbc = make_rbc(agg, slice(256, 384))
                    if gate:
                        aggn = []
                        for b in range(2):
                            t = npool.tile([128, 128], BF, tag=f"aggn{b}",
                                           name="aggn")
                            nc.vector.tensor_tensor(
                                out=t[:], in0=agg[:, b * 128:(b + 1) * 128],
                                in1=rbc[:], op=alu.mult)
                            aggn.append(t)
                        h0 = ps_tile("ch", "h0ps")
                        for b in range(2):
                            for k in range(2):
                                nc.tensor.matmul(
                                    h0[:, b * 256:b * 256 + 128],
                                    lhsT=W(f"gl2T_{k}{b}"), rhs=aggn[k][:],
                                    start=(b == 0 and k == 0),
                                    stop=(b == 1 and k == 1),
                                    skip_group_check=True)
                        for b in range(2):
                            t = npool.tile([128, 128], BF, tag="tb", name="tb")
                            nc.vector.tensor_copy(t[:], h0[:, b * 256:b * 256 + 128])
                            elu_into(hTg[:, b * 512 + wi * 128:
                                         b * 512 + wi * 128 + 128],
                                     t, F(f"gb2{b}"), 128)
                    else:
                        for b in range(2):
                            t = npool.tile([128, 128], F32, tag="tb", name="tb")
                            nc.vector.tensor_tensor(
                                out=t[:], in0=agg[:, b * 128:(b + 1) * 128],
                                in1=rbc[:], op=alu.mult)
                            elu_into(hTg[:, b * 512 + wi * 128:
                                         b * 512 + wi * 128 + 128],
                                     t, F(f"ab{li - 1}{b}"), 128)

                # ---- GRU over the 4-window group ----
                gru_batched(li, hTg, 512, slice(grp * 512, (grp + 1) * 512), 512)
                # ---- next-layer tables ----
                for wi in range(4):
                    table_rows(li, grp * 4 + wi, last=(li == 3))
                if has_ag:
                    for q in ag_quarters(grp):
                        launch_ag(dst_tab, q)

        edge_layer(0, 'gate', tabs[0], True)
        edge_layer(1, 'atom', tabs[1], True)
        edge_layer(2, 'atom', tabs[0], True)
        edge_layer(3, 'atom', tabs[1], False)

        # ================= mol phase =================
        # readout: out0 = relu(segment_sum(x))
        ro = [ps_tile("agg", "ro0"), ps_tile("agg", "ro1")]
        for w in range(NWIN):
            wsl = slice(w * 128, (w + 1) * 128)
            xr = mpool.tile([128, H], BF, tag="xr")
            nc.sync.dma_start(xr[:], xld.ap()[wsl, :])
            selg = npool.tile([128, NG], BF, tag="selg", name="selg")
            nc.vector.tensor_scalar(out=selg[:], in0=W("iota256")[:, 0:NG],
                                    scalar1=glocT[:, w:w + 1], scalar2=None,
                                    op0=alu.is_equal)
            for b in range(2):
                nc.tensor.matmul(ro[b][:, 0:NG],
                                 lhsT=xr[:, b * 128:(b + 1) * 128],
                                 rhs=selg[:], start=(w == 0),
                                 stop=(w == NWIN - 1))
        for b in range(2):
            nc.vector.tensor_scalar(out=outTs[b][:], in0=ro[b][:, 0:NG],
                                    scalar1=0.0, scalar2=None, op0=alu.max)

        for step in range(3):
            # hr rows for graphs  [2 x (128 g, 256 f)]
            hrm = []
            for gb in range(2):
                p = ps_tile("ch", "hrmps")
                for k in range(2):
                    nc.tensor.matmul(p[:, 0:256],
                                     lhsT=outTs[k][:, gb * 128:(gb + 1) * 128],
                                     rhs=W("WrTr3")[:, k * 256:(k + 1) * 256],
                                     start=(k == 0), stop=(k == 1))
                t = upool.tile([128, 256], BF, tag=f"hrm{gb}", name="hrm")
                nc.vector.tensor_copy(t[:], p[:, 0:256])
                hrm.append(t)

            agA = ps_tile("agg", "agA")   # agm0 @0:256, sums @256:512
            agB = ps_tile("agg", "agB")   # agm1 @0:256
            nw4 = (NWIN + 3) // 4
            for g4 in range(nw4):
                wlist = range(g4 * 4, min((g4 + 1) * 4, NWIN))
                ecm = spool.tile([128, 4], F32, tag="ecm", name="ecm")
                tls = []
                for i, w in enumerate(wlist):
                    wsl = slice(w * 128, (w + 1) * 128)
                    selgTw = mpool.tile([128, 256], BF, tag="selgTw")
                    nc.sync.dma_start(selgTw[:], selgTblk.ap()[w])
                    ch = ps_tile("ch", "chps")
                    for gb in range(2):
                        nc.tensor.matmul(ch[:, 0:256],
                                         lhsT=selgTw[:, gb * 128:(gb + 1) * 128],
                                         rhs=hrm[gb][:], start=(gb == 0),
                                         stop=False)
                    nc.tensor.matmul(ch[:, 0:256], lhsT=ident,
                                     rhs=hrR[:, w * 256:(w + 1) * 256],
                                     start=False, stop=True)
                    tsb = npool.tile([128, H], BF, tag="tsb", name="tsbm")
                    nc.vector.tensor_copy(tsb[:], ch[:, 0:256])
                    tlm = npool.tile([128, H], BF, tag="tls", name="tlm")
                    nc.vector.scalar_tensor_tensor(
                        out=tlm[:], in0=tsb[:], scalar=0.01, in1=tsb[:],
                        op0=alu.mult, op1=alu.max)
                    escr = npool.tile([128, H], BF, tag="escr", name="escrm")
                    nc.vector.scalar_tensor_tensor(
                        out=escr[:], in0=tlm[:], scalar=1.0, in1=W("attm"),
                        op0=alu.mult, op1=alu.mult,
                        accum_out=ecm[:, i:i + 1])
                exm = spool.tile([128, 4], F32, tag="exm", name="exm")
                nc.scalar.activation(exm[:, 0:len(list(wlist))],
                                     ecm[:, 0:len(list(wlist))], act.Exp)
                for i, w in enumerate(wlist):
                    selwm = npool.tile([128, NG], BF, tag="selwm", name="selwm")
                    nc.vector.tensor_scalar(
                        out=selwm[:], in0=W("iota256")[:, 0:NG],
                        scalar1=glocT[:, w:w + 1], scalar2=exm[:, i:i + 1],
                        op0=alu.is_equal, op1=alu.mult)
                    first = (w == 0)
                    last = (w == NWIN - 1)
                    nc.tensor.matmul(agA[:, 0:NG],
                                     lhsT=hrR[:, w * 256:w * 256 + 128],
                                     rhs=selwm[:], start=first, stop=last,
                                     skip_group_check=True)
                    nc.tensor.matmul(agB[:, 0:NG],
                                     lhsT=hrR[:, w * 256 + 128:(w + 1) * 256],
                                     rhs=selwm[:], start=first, stop=last,
                                     skip_group_check=True)
                    nc.tensor.matmul(agA[0:1, NG:2 * NG], lhsT=ones_col,
                                     rhs=selwm[:], start=False, stop=last,
                                     skip_group_check=True)

            # mol epilogue
            sums_sb = spool.tile([1, NG], F32, tag="sumsm", name="sumsm")
            nc.vector.tensor_scalar(out=sums_sb[:], in0=agA[0:1, NG:2 * NG],
                                    scalar1=EPS, scalar2=None, op0=alu.max)
            rrow = spool.tile([1, NG], F32, tag="rrowm", name="rrowm")
            nc.vector.reciprocal(rrow[:], sums_sb[:])
            bps = ps_tile("ch", "bcpsm")
            nc.tensor.matmul(bps[:, 0:NG], lhsT=ones1f[:], rhs=rrow[:],
                             start=True, stop=True)
            rbcm = upool.tile([128, NG], F32, tag="rbcm", name="rbcm", bufs=1)
            nc.vector.tensor_copy(rbcm[:], bps[:, 0:NG])
            hTmg = upool.tile([128, 2 * NG], BF, tag="hTmg", name="hTmg", bufs=1)
            for b in range(2):
                ag_b = agA if b == 0 else agB
                t = upool.tile([128, NG], F32, tag="tbm", name="tbm", bufs=1)
                nc.vector.tensor_tensor(out=t[:], in0=ag_b[:, 0:NG],
                                        in1=rbcm[:], op=alu.mult)
                elu_into(hTmg[:, b * NG:(b + 1) * NG], t, F(f"molb{b}"), NG)
            gru_batched(4, hTmg, NG, slice(0, NG), NG)

        # ================= MLP head =================
        o1ps = ps_tile("ch", "o1ps")
        for k in range(2):
            nc.tensor.matmul(o1ps[:, 0:NG], lhsT=W("w1T")[:, k * 128:(k + 1) * 128],
                             rhs=outTs[k][:], start=(k == 0), stop=(k == 1))
        o1 = npool.tile([128, NG], BF, tag="o1", name="o1")
        nc.scalar.activation(o1[:], o1ps[:, 0:NG], act.Relu, bias=F("b1"))
        o2ps = ps_tile("gp", "o2ps")
        nc.tensor.matmul(o2ps[0:64, 0:NG], lhsT=W("w2T"), rhs=o1[:],
                         start=True, stop=True)
        o2 = npool.tile([65, NG], BF, tag="o2", name="o2")
        nc.vector.memset(o2[64:65, :], 1.0)
        nc.scalar.activation(o2[0:64, :], o2ps[0:64, 0:NG], act.Relu,
                             bias=F("b2_")[0:64, :])
        o3ps = ps_tile("gp", "o3ps")
        nc.tensor.matmul(o3ps[0:1, 0:NG], lhsT=w3T[:], rhs=o2[:],
                         start=True, stop=True)
        o3 = spool.tile([1, NG], F32, tag="o3", name="o3")
        nc.vector.tensor_copy(o3[:], o3ps[0:1, 0:NG])
        nc.sync.dma_start(y.ap()[:, 0:NG], o3[:])

    return nc


def make_core_inputs(P, inputs, ci, wpb, wpf):
    c = P['cores'][ci]
    NLOC, NWIN, CW = P['NLOC'], P['NWIN'], P['CW']
    x = np.asarray(inputs['x'], np.float32)
    xinT = np.zeros((65, NLOC), np.float32)
    xinT[:64, :c['nn']] = x[c['ns']:c['ne']].T
    xinT[64, :] = 1.0
    ea = np.asarray(inputs['edge_attr'], np.float32)
    ea_perm = np.where(c['sl_fill'][:, None], ea[c['sl_edge']], 0.0)
    eaT2 = ea_perm.reshape(NWIN, CW * 128, 16).transpose(0, 2, 1).astype(BF16)
    return dict(
        xinT=xinT.astype(BF16),
        srcblk=c['srcblk'], dlblk=c['dlblk'],
        selTblk=c['selTblk'], eaT2=eaT2,
        glocT=c['glocT'], selgTblk=c['selgT'],
        wpb=wpb, wpf=wpf,
        lin1T=np.concatenate([np.asarray(inputs['lin1_w'], np.float32).T,
                              np.asarray(inputs['lin1_b'], np.float32)[None, :]],
                             0).astype(BF16),
        W2T=np.asarray(inputs['g_lin1_w'], np.float32)[:, 256:].T.copy().astype(BF16),
        w3T=np.concatenate([np.asarray(inputs['mlp_w3'], np.float32).T,
                            np.asarray(inputs['mlp_b3'], np.float32).reshape(1, 1)],
                           0).astype(BF16),
    )


_CACHE = {}
LAST_EXEC_NS = None

def kernel(**inputs):
    inputs = dict(inputs)
    edge_index = np.asarray(inputs['edge_index']).astype(np.int64)
    batch = np.asarray(inputs['batch']).astype(np.int64)
    n_cores = 8
    G = 2048
    P = preprocess(edge_index, batch, n_cores=n_cores, G=G, CW=5)
    key = (P['NLOC'], P['NWIN'], P['CW'], P['GPC'])
    if key not in _CACHE:
        _CACHE[key] = build_kernel(P['NLOC'], P['NWIN'], P['CW'], P['GPC'],
                                   n_cores)
    nc = _CACHE[key]
    wpb, wpf = make_wpacks(inputs)
    ins = [make_core_inputs(P, inputs, ci, wpb, wpf) for ci in range(n_cores)]
    from concourse.bass_utils import run_bass_kernel_spmd
    trace = bool(os.environ.get('BASS_KERNEL_TRACE'))
    res = run_bass_kernel_spmd(nc, ins, list(range(n_cores)), trace=trace)
    if trace:
        global LAST_EXEC_NS
        LAST_EXEC_NS = res.exec_time_ns
    yv = np.concatenate([np.asarray(res.results[c]['y'][0, :P['GPC']],
                                    np.float32) for c in range(n_cores)])
    return yv.reshape(G, 1).astype(np.float32)


# revision 20
# speedup vs baseline: 1.1712x; 1.1712x over previous
"""AttentiveFP forward pass as a Bass/Tile kernel on 8 Trainium2 NeuronCores.

v2: bf16 matmuls (4x PE rate + FWL weight loads), host-precomputed edge
selection matrices, aggregation of the gathered hl rows directly (no hrT
reconstruction), GRU batched over 512-node column groups with stationary
weights, Exp-only ACT function in the chunk sweep (Lrelu/Relu/Elu built from
DVE min/max + exp), column-wise reciprocal for the softmax normalizer, and
double-buffered node tables with quarter-chunked AllGathers overlapping
compute.

Data-parallel by graph blocks (256 graphs/core); edges assigned to the core
owning their dst node; per-core windowed segment-softmax aggregation via
selection-matrix matmuls on the PE; per-edge source rows fetched with
indirect DMA gathers from the AllGathered table.
"""
import sys, os
sys.path.insert(0, '/opt/trn_rl_repo')
import numpy as np
import ml_dtypes
from contextlib import ExitStack

import concourse.bass as bass
import concourse.mybir as mybir
import concourse.tile as tile
from concourse.bass import IndirectOffsetOnAxis
from concourse.mybir import AluOpType as alu, ActivationFunctionType as act

BF16 = ml_dtypes.bfloat16
BF = mybir.dt.bfloat16
F32 = mybir.dt.float32
I32 = mybir.dt.int32
EPS = 1e-30

# ---------------- walrus sync-wait splitting ----------------
MAX_WAITS = 1

def split_waits(nc):
    eng_map = nc.engines
    for bbname, bassbb in nc.bb_map.items():
        insts = bassbb.bb.instructions
        i = 0
        while i < len(insts):
            inst = insts[i]
            si = inst.sync_info
            if si is not None and si.on_wait is not None and len(si.on_wait) > MAX_WAITS:
                waits = list(si.on_wait)
                si.on_wait = waits[-MAX_WAITS:]
                rest = waits[:-MAX_WAITS]
                for j in range(0, len(rest), MAX_WAITS):
                    eng = eng_map[inst.engine]
                    nop = eng.nop(nofuse=True)
                    nop_inst = nop.ins
                    for obb in nc.bb_map.values():
                        lst = obb.bb.instructions
                        for k in range(len(lst) - 1, -1, -1):
                            if lst[k].name == nop_inst.name:
                                del lst[k]
                                break
                    nsi = nop_inst.sync_info
                    chunk = rest[j:j + MAX_WAITS]
                    if nsi is None:
                        nop_inst.sync_info = mybir.SyncInfo(on_wait=chunk, on_update=[])
                    else:
                        nsi.on_wait = chunk
                    insts.insert(i, nop_inst)
                    i += 1
            i += 1


class TileContextFixed(tile.TileContext):
    def __exit__(self, *args):
        r = super().__exit__(*args)
        split_waits(self.nc)
        return r


def preprocess(edge_index, batch, n_cores=8, G=2048, CW=5):
    src = np.asarray(edge_index[0]).astype(np.int64)
    dst = np.asarray(edge_index[1]).astype(np.int64)
    batch = np.asarray(batch).astype(np.int64)
    N = batch.shape[0]
    GPC = G // n_cores
    gstart = np.searchsorted(batch, np.arange(0, G + 1, GPC))
    ncounts = np.diff(gstart)
    NLOC = int(np.ceil(ncounts.max() / 512) * 512)
    NWIN = NLOC // 128
    NLQ = NLOC // 4

    node_owner = np.searchsorted(gstart, np.arange(N), side='right') - 1
    loc = np.arange(N) - gstart[node_owner]
    q = loc // NLQ
    ag_row = (q * (n_cores * NLQ) + node_owner * NLQ + (loc % NLQ)).astype(np.int64)
    owner = node_owner[dst]
    ar128 = np.arange(128)

    cores = []
    for c in range(n_cores):
        ns, ne = int(gstart[c]), int(gstart[c + 1])
        nn = ne - ns
        m = owner == c
        eidx = np.nonzero(m)[0]
        dl = dst[eidx] - ns
        order = np.argsort(dl, kind='stable')
        eidx = eidx[order]; dl = dl[order]
        win = dl // 128
        counts = np.bincount(win, minlength=NWIN)
        assert counts.max() <= CW * 128, f"window overflow {counts.max()}"
        pos = np.concatenate([[0], np.cumsum(counts)])[:-1]
        within = np.arange(len(dl)) - pos[win]
        slots = (win * CW * 128 + within).astype(np.int64)

        sl_src = np.zeros(NWIN * CW * 128, np.int32)
        sl_dl = np.full(NWIN * CW * 128, -1.0, np.float32)
        sl_edge = np.zeros(NWIN * CW * 128, np.int64)
        sl_fill = np.zeros(NWIN * CW * 128, bool)
        sl_src[slots] = ag_row[src[eidx]]
        sl_dl[slots] = (dl % 128).astype(np.float32)
        sl_edge[slots] = eidx
        sl_fill[slots] = True

        srcblk = sl_src.reshape(NWIN, CW, 128).transpose(0, 2, 1).copy()
        dlblk = sl_dl.reshape(NWIN, CW, 128).transpose(0, 2, 1).copy()
        # selT [w, d, c*128+p] = 1 if dstloc(w,p,c)==d
        selw = (dlblk.transpose(0, 2, 1)[:, :, None, :] ==
                ar128[None, None, :, None])                  # [w, c, d, p]
        selTblk = selw.transpose(0, 2, 1, 3).reshape(
            NWIN, 128, CW * 128).astype(BF16)

        gloc = np.full(NLOC, -1.0, np.float32)
        gloc[:nn] = (batch[ns:ne] - c * GPC).astype(np.float32)
        glocT = gloc.reshape(NWIN, 128).T.copy()             # [128, NWIN]
        gw = gloc.reshape(NWIN, 128)
        selgT = np.zeros((NWIN, 128, 256), BF16)
        for gb in range(2):
            selgT[:, :, gb * 128:(gb + 1) * 128] = (
                gw[:, None, :] == (gb * 128 + ar128)[None, :, None])

        cores.append(dict(ns=ns, ne=ne, nn=nn,
                          srcblk=srcblk, dlblk=dlblk, selTblk=selTblk,
                          glocT=glocT, selgT=selgT,
                          sl_edge=sl_edge, sl_fill=sl_fill))
    return dict(cores=cores, gstart=gstart, NLOC=NLOC, NWIN=NWIN, NLQ=NLQ,
                CW=CW, GPC=GPC, n_cores=n_cores)


def wpb_layout():
    """bf16 pack: name -> (off, cols)."""
    L = {}
    off = 0
    def add(name, cols):
        nonlocal off
        L[name] = (off, cols)
        off += cols
    add("iota_sq", 128)
    add("iota256", 256)
    add("ones_col", 1)
    add("ident", 128)
    add("attg", 256)
    for l in range(3):
        add(f"atta{l}", 256)
    add("attm", 256)
    for b in range(2):
        add(f"W1A{b}", 257)
    for i in range(4):
        add(f"WWr{i}", 1024)   # k-chunk k at k*512: [WlT_k(256) | (WrT_k or id_k)(256)]
    add("WrTM", 512)           # mol_Wr.T k-chunks
    for k in range(2):
        for b in range(2):
            add(f"gl2T_{k}{b}", 128)
    for g in range(5):
        for j in range(12):
            for b in range(2):
                add(f"gru{g}_w{j}{b}", 128)
    add("w1T", 256)
    add("w2T", 64)
    return L, off


def wpf_layout():
    """f32 pack (biases): name -> (off, cols)."""
    L = {}
    off = 0
    def add(name, cols):
        nonlocal off
        L[name] = (off, cols)
        off += cols
    for g in range(5):
        for j in range(4):
            for b in range(2):
                add(f"gru{g}_b{j}{b}", 1)
    for l in range(3):
        for b in range(2):
            add(f"ab{l}{b}", 1)
    for b in range(2):
        add(f"gb2{b}", 1)
    for b in range(2):
        add(f"molb{b}", 1)
    add("b1", 1)
    add("b2_", 1)
    return L, off


def make_wpacks(inp):
    Lb, WB = wpb_layout()
    Lf, WF = wpf_layout()
    Wb = np.zeros((128, WB), np.float32)
    Wf = np.zeros((128, WF), np.float32)

    def putb(name, arr):
        off, cols = Lb[name]
        assert arr.shape == (128, cols), (name, arr.shape, cols)
        Wb[:, off:off + cols] = arr

    def putf(name, arr):
        off, cols = Lf[name]
        assert arr.shape == (128, cols), (name, arr.shape, cols)
        Wf[:, off:off + cols] = arr

    putb("iota_sq", np.tile(np.arange(128, dtype=np.float32), (128, 1)))
    putb("iota256", np.tile(np.arange(256, dtype=np.float32), (128, 1)))
    putb("ones_col", np.ones((128, 1), np.float32))
    I = np.eye(128, dtype=np.float32)
    putb("ident", I)
    putb("attg", np.tile(np.asarray(inp['g_att_l'], np.float32), (128, 1)))
    for l in range(3):
        putb(f"atta{l}", np.tile(np.asarray(inp['atom_att'][l], np.float32), (128, 1)))
    putb("attm", np.tile(np.asarray(inp['mol_att'], np.float32), (128, 1)))
    W1T = np.asarray(inp['g_lin1_w'], np.float32)[:, :256].T     # [256 in, 256 out]
    attr = np.asarray(inp['g_att_r'], np.float32).reshape(2, 128).T  # [128, 2]
    for b in range(2):
        putb(f"W1A{b}", np.concatenate(
            [W1T[b * 128:(b + 1) * 128], attr[:, b:b + 1]], axis=1))
    Wls = [np.asarray(inp['atom_Wl'][0]), np.asarray(inp['atom_Wl'][1]),
           np.asarray(inp['atom_Wl'][2]), np.asarray(inp['mol_Wl'])]
    Wrs = [np.asarray(inp['atom_Wr'][0]), np.asarray(inp['atom_Wr'][1]),
           np.asarray(inp['atom_Wr'][2])]
    Z = np.zeros((128, 128), np.float32)
    for i in range(4):
        WlT = Wls[i].T.astype(np.float32)
        blocks = []
        for k in range(2):
            if i < 3:
                WrT = Wrs[i].T.astype(np.float32)
                right = WrT[k * 128:(k + 1) * 128]
            else:
                right = np.concatenate([I, Z], 1) if k == 0 else \
                        np.concatenate([Z, I], 1)
            blocks.append(np.concatenate([WlT[k * 128:(k + 1) * 128], right], 1))
        putb(f"WWr{i}", np.concatenate(blocks, 1))
    WrTM = np.asarray(inp['mol_Wr'], np.float32).T
    putb("WrTM", np.concatenate([WrTM[0:128], WrTM[128:256]], 1))
    g2T = np.asarray(inp['g_lin2_w'], np.float32).T              # [k, out]
    for k in range(2):
        for b in range(2):
            putb(f"gl2T_{k}{b}", g2T[k * 128:(k + 1) * 128, b * 128:(b + 1) * 128])
    grus = [('gru0_wih', 'gru0_whh', 'gru0_bih', 'gru0_bhh', None),
            ('agru_wih', 'agru_whh', 'agru_bih', 'agru_bhh', 0),
            ('agru_wih', 'agru_whh', 'agru_bih', 'agru_bhh', 1),
            ('agru_wih', 'agru_whh', 'agru_bih', 'agru_bhh', 2),
            ('mgru_wih', 'mgru_whh', 'mgru_bih', 'mgru_bhh', None)]
    for g, (wi, wh, bi, bh, l) in enumerate(grus):
        wih = np.asarray(inp[wi] if l is None else inp[wi][l], np.float32)
        whh = np.asarray(inp[wh] if l is None else inp[wh][l], np.float32)
        bih = np.asarray(inp[bi] if l is None else inp[bi][l], np.float32)
        bhh = np.asarray(inp[bh] if l is None else inp[bh][l], np.float32)
        wihT = wih.T    # [256 k, 768]
        whhT = whh.T
        # j: 0,1 wih-r (input side); 2,3 whh-r (hidden side); 4..7 z; 8,9 wih-n; 10,11 whh-n
        for k in range(2):
            for b in range(2):
                ks, bs = slice(k * 128, (k + 1) * 128), slice(b * 128, (b + 1) * 128)
                putb(f"gru{g}_w{0 + k}{b}", wihT[ks, 0:256][:, bs])
                putb(f"gru{g}_w{2 + k}{b}", whhT[ks, 0:256][:, bs])
                putb(f"gru{g}_w{4 + k}{b}", wihT[ks, 256:512][:, bs])
                putb(f"gru{g}_w{6 + k}{b}", whhT[ks, 256:512][:, bs])
                putb(f"gru{g}_w{8 + k}{b}", wihT[ks, 512:768][:, bs])
                putb(f"gru{g}_w{10 + k}{b}", whhT[ks, 512:768][:, bs])
        br = (bih[0:256] + bhh[0:256]).reshape(2, 128).T
        bz = (bih[256:512] + bhh[256:512]).reshape(2, 128).T
        bin_ = bih[512:768].reshape(2, 128).T
        bhn = bhh[512:768].reshape(2, 128).T
        for j, arr in enumerate([br, bz, bin_, bhn]):
            for b in range(2):
                putf(f"gru{g}_b{j}{b}", arr[:, b:b + 1])
    ab = np.asarray(inp['atom_bias'], np.float32)
    for l in range(3):
        for b in range(2):
            putf(f"ab{l}{b}", ab[l].reshape(2, 128).T[:, b:b + 1])
    gb2 = np.asarray(inp['g_bias'], np.float32).reshape(2, 128).T
    molb = np.asarray(inp['mol_bias'], np.float32).reshape(2, 128).T
    for b in range(2):
        putf(f"gb2{b}", gb2[:, b:b + 1])
        putf(f"molb{b}", molb[:, b:b + 1])
    putf("b1", np.asarray(inp['mlp_b1'], np.float32).reshape(128, 1))
    putf("b2_", np.pad(np.asarray(inp['mlp_b2'], np.float32), (0, 64)).reshape(128, 1))
    w1T = np.asarray(inp['mlp_w1'], np.float32).T                # [256, 128]
    putb("w1T", np.concatenate([w1T[0:128], w1T[128:256]], 1))
    putb("w2T", np.asarray(inp['mlp_w2'], np.float32).T)         # [128, 64]
    return Wb.astype(BF16), Wf


def build_kernel(NLOC, NWIN, CW, NG, n_cores):
    H = 256
    NLQ = NLOC // 4
    NGRP = NWIN // 4
    NWQ = NWIN // 4            # windows per AG quarter

    nc = bass.Bass(num_devices=n_cores)
    Lb, WB = wpb_layout()
    Lf, WF = wpf_layout()

    def dram_in(name, shape, dt=BF):
        return nc.dram_tensor(name, list(shape), dt, kind="ExternalInput")

    xinT = dram_in("xinT", [65, NLOC])
    srcblk = dram_in("srcblk", [NWIN, 128, CW], I32)
    dlblk = dram_in("dlblk", [NWIN, 128, CW], F32)
    selTblk = dram_in("selTblk", [NWIN, 128, CW * 128])
    eaT2 = dram_in("eaT2", [NWIN, 16, CW * 128])
    glocTd = dram_in("glocT", [128, NWIN], F32)
    selgTblk = dram_in("selgTblk", [NWIN, 128, 256])
    wpbd = dram_in("wpb", [128, WB])
    wpfd = dram_in("wpf", [128, WF], F32)
    lin1Td = dram_in("lin1T", [65, 256])
    W2Td = dram_in("W2T", [16, 256])
    w3Td = dram_in("w3T", [65, 1])

    y = nc.dram_tensor("y", [1, 256], F32, kind="ExternalOutput")

    cc_in = nc.dram_tensor("cc_in", [NLOC, H], BF)
    tabs = [nc.dram_tensor(f"tab{i}", [n_cores * NLOC, H], BF, addr_space="Shared")
            for i in range(2)]
    xld = nc.dram_tensor("xld", [NLOC, H], BF)

    with TileContextFixed(nc) as tc, ExitStack() as ctx:
        wpool = ctx.enter_context(tc.tile_pool(name="weights", bufs=1))
        mpool = ctx.enter_context(tc.tile_pool(name="meta", bufs=3))
        gpool = ctx.enter_context(tc.tile_pool(name="gath", bufs=8))
        vpool = ctx.enter_context(tc.tile_pool(name="vals", bufs=8))
        npool = ctx.enter_context(tc.tile_pool(name="node", bufs=3))
        spool = ctx.enter_context(tc.tile_pool(name="small", bufs=4))
        upool = ctx.enter_context(tc.tile_pool(name="gru", bufs=2))
        pp = ctx.enter_context(tc.tile_pool(name="ps", bufs=1, space="PSUM"))

        wpb = wpool.tile([128, WB], BF, tag="wpb")
        nc.sync.dma_start(wpb[:], wpbd.ap())
        wpf = wpool.tile([128, WF], F32, tag="wpf")
        nc.sync.dma_start(wpf[:], wpfd.ap())

        def W(name):
            off, cols = Lb[name]
            return wpb[:, off:off + cols]

        def F(name):
            off, cols = Lf[name]
            return wpf[:, off:off + cols]

        iota_sq = W("iota_sq")
        ident = W("ident")
        ones_col = W("ones_col")
        lin1T = wpool.tile([65, 256], BF, tag="lin1T")
        nc.sync.dma_start(lin1T[:], lin1Td.ap())
        W2T = wpool.tile([16, 256], BF, tag="W2T")
        nc.sync.dma_start(W2T[:], W2Td.ap())
        w3T = wpool.tile([65, 1], BF, tag="w3T")
        nc.sync.dma_start(w3T[:], w3Td.ap())
        ones1 = wpool.tile([1, 128], BF, tag="ones1")
        nc.vector.memset(ones1[:], 1.0)
        onesq = wpool.tile([1, 1], BF, tag="onesq")
        nc.vector.memset(onesq[:], 1.0)
        glocT = wpool.tile([128, NWIN], F32, tag="glocT")
        nc.sync.dma_start(glocT[:], glocTd.ap())

        # persistent node-state tiles
        xts = [wpool.tile([128, NLOC], BF, tag=f"xT{b}", name=f"xT{b}") for b in range(2)]
        hrR = wpool.tile([128, NWIN * 256], BF, tag="hrR")
        wc = wpool.tile([128, NWIN], BF, tag="wc")
        outTs = [wpool.tile([128, NG], BF, tag=f"outT{b}", name=f"outT{b}") for b in range(2)]

        def ps_tile(tag, name):
            return pp.tile([128, 512], F32, tag=tag, name=name, bufs={
                "agg": 2, "ch": 2, "gp": 3}[tag])

        def launch_ag(dst_tab, q):
            rs = cc_in.ap()[q * NLQ:(q + 1) * NLQ, :]
            os_ = dst_tab.ap()[q * n_cores * NLQ:(q + 1) * n_cores * NLQ, :]
            nc.gpsimd.collective_compute(
                "AllGather", alu.bypass,
                replica_groups=[list(range(n_cores))],
                ins=[rs], outs=[os_])

        def ag_quarters(grp):
            return [q for q in range(4) if ((q + 1) * NWQ - 1) // 4 == grp]

        # ---------- softmax normalizer: row sums -> rbc [128,128] ----------
        def make_rbc(agg, sums_cols):
            """agg bank holds row [1,128] of exp-sums at sums_cols; returns
            rbc sbuf tile [128,128] bf16 with 1/sum broadcast down columns."""
            sums_sb = spool.tile([1, 128], BF, tag="sums_sb", name="sums_sb")
            nc.vector.tensor_scalar(out=sums_sb[:], in0=agg[0:1, sums_cols],
                                    scalar1=EPS, scalar2=None, op0=alu.max)
            cps = ps_tile("ch", "colps")
            nc.tensor.matmul(cps[:, 0:1], lhsT=sums_sb[:], rhs=onesq[:],
                             start=True, stop=True)
            recipc = spool.tile([128, 1], BF, tag="recipc", name="recipc")
            with nc.allow_low_precision(reason="bf16 softmax normalizer"):
                nc.vector.reciprocal(recipc[:], cps[:, 0:1])
            rps = ps_tile("ch", "rowps")
            nc.tensor.matmul(rps[0:1, 0:128], lhsT=recipc[:], rhs=ident,
                             start=True, stop=True)
            rrow = spool.tile([1, 128], BF, tag="rrow", name="rrow")
            nc.vector.tensor_copy(rrow[:], rps[0:1, 0:128])
            bps = ps_tile("ch", "bcps")
            nc.tensor.matmul(bps[:, 0:128], lhsT=ones1[:], rhs=rrow[:],
                             start=True, stop=True)
            rbc = npool.tile([128, 128], BF, tag="rbc", name="rbc")
            nc.vector.tensor_copy(rbc[:], bps[:, 0:128])
            return rbc

        def elu_into(dst_ap, t, bias_ap, fd):
            """dst = elu(t + bias); t is sbuf bf16 [128, fd]."""
            m = npool.tile([128, fd], BF, tag=f"elu_m{fd}", name="elum")
            nc.vector.tensor_scalar(out=m[:], in0=t[:], scalar1=bias_ap,
                                    scalar2=0.0, op0=alu.add, op1=alu.min)
            r = npool.tile([128, fd], BF, tag=f"elu_r{fd}", name="elur")
            nc.vector.tensor_scalar(out=r[:], in0=t[:], scalar1=bias_ap,
                                    scalar2=0.0, op0=alu.add, op1=alu.max)
            e = npool.tile([128, fd], BF, tag=f"elu_e{fd}", name="elue")
            nc.scalar.activation(e[:], m[:], act.Exp)
            nc.vector.scalar_tensor_tensor(out=dst_ap, in0=e[:], scalar=-1.0,
                                           in1=r[:], op0=alu.add, op1=alu.add)

        # ---------------- batched GRU ----------------
        def gru_batched(g, hTg, kspan, xsl, ncols):
            """hTg: [128, 2*kspan] bf16 (input feature half k at cols k*kspan);
            hidden state = xts-like tiles given by closure target `gxts`;
            writes relu'd new state back into gxts[b][:, xsl]."""
            gxts = xts if g < 4 else outTs

            def gate_ps(b, jh, jx, name):
                p = ps_tile("gp", name)
                nmm = (2 if jh is not None else 0) + (2 if jx is not None else 0)
                i = 0
                for k in range(2):
                    if jh is not None:
                        nc.tensor.matmul(p[:, 0:ncols],
                                         lhsT=W(f"gru{g}_w{jh + k}{b}"),
                                         rhs=hTg[:, k * kspan:k * kspan + ncols],
                                         start=(i == 0), stop=(i == nmm - 1))
                        i += 1
                for k in range(2):
                    if jx is not None:
                        nc.tensor.matmul(p[:, 0:ncols],
                                         lhsT=W(f"gru{g}_w{jx + k}{b}"),
                                         rhs=gxts[k][:, xsl],
                                         start=(i == 0), stop=(i == nmm - 1))
                        i += 1
                return p

            rps = [gate_ps(b, 0, 2, "rps") for b in range(2)]
            r = []
            for b in range(2):
                t = upool.tile([128, ncols], BF, tag="gru_r", name="grur")
                nc.scalar.activation(t[:], rps[b][:, 0:ncols], act.Sigmoid,
                                     bias=F(f"gru{g}_b0{b}"))
                r.append(t)
            zps = [gate_ps(b, 4, 6, "zps") for b in range(2)]
            z = []
            for b in range(2):
                t = upool.tile([128, ncols], BF, tag="gru_z", name="gruz")
                nc.scalar.activation(t[:], zps[b][:, 0:ncols], act.Sigmoid,
                                     bias=F(f"gru{g}_b1{b}"))
                z.append(t)
            hps = [gate_ps(b, None, 10, "hps") for b in range(2)]
            t1 = []
            for b in range(2):
                t = upool.tile([128, ncols], BF, tag="gru_t1", name="grut1")
                nc.vector.scalar_tensor_tensor(out=t[:], in0=hps[b][:, 0:ncols],
                                               scalar=F(f"gru{g}_b3{b}"),
                                               in1=r[b][:], op0=alu.add,
                                               op1=alu.mult)
                t1.append(t)
            ips = [gate_ps(b, 8, None, "ips") for b in range(2)]
            t2 = []
            for b in range(2):
                t = upool.tile([128, ncols], BF, tag="gru_t2", name="grut2")
                nc.vector.tensor_tensor(out=t[:], in0=ips[b][:, 0:ncols],
                                        in1=t1[b][:], op=alu.add)
                t2.append(t)
            n = []
            for b in range(2):
                t = upool.tile([128, ncols], BF, tag="gru_n", name="grun")
                nc.scalar.activation(t[:], t2[b][:], act.Tanh,
                                     bias=F(f"gru{g}_b2{b}"))
                n.append(t)
            for b in range(2):
                d = upool.tile([128, ncols], BF, tag="gru_d", name="grud")
                nc.vector.tensor_tensor(out=d[:], in0=gxts[b][:, xsl],
                                        in1=n[b][:], op=alu.subtract)
                zd = upool.tile([128, ncols], BF, tag="gru_zd", name="gruzd")
                nc.vector.tensor_tensor(out=zd[:], in0=z[b][:], in1=d[:],
                                        op=alu.mult)
                s = upool.tile([128, ncols], BF, tag="gru_s", name="grus")
                nc.vector.tensor_tensor(out=s[:], in0=n[b][:], in1=zd[:],
                                        op=alu.add)
                nc.vector.tensor_scalar(out=gxts[b][:, xsl], in0=s[:],
                                        scalar1=0.0, scalar2=None, op0=alu.max)

        # ---------------- table production (per window) ----------------
        def table_rows(li, w, last):
            wsl = slice(w * 128, (w + 1) * 128)
            p = ps_tile("ch", "tabps")
            for k in range(2):
                nc.tensor.matmul(p[:, 0:512], lhsT=xts[k][:, wsl],
                                 rhs=W(f"WWr{li}")[:, k * 512:(k + 1) * 512],
                                 start=(k == 0), stop=(k == 1))
            if last:
                # hl_m rows for the mol phase live in hrR; x rows to xld
                nc.vector.tensor_copy(hrR[:, w * 256:(w + 1) * 256], p[:, 0:256])
                xsb = npool.tile([128, 256], BF, tag="cp", name="xsb")
                nc.vector.tensor_copy(xsb[:], p[:, 256:512])
                nc.sync.dma_start(xld.ap()[wsl, :], xsb[:])
            else:
                hsb = npool.tile([128, 256], BF, tag="cp", name="hsb")
                nc.vector.tensor_copy(hsb[:], p[:, 0:256])
                nc.sync.dma_start(cc_in.ap()[wsl, :], hsb[:])
                nc.vector.tensor_copy(hrR[:, w * 256:(w + 1) * 256], p[:, 256:512])

        # ================= P0: input projection + u table =================
        for grp in range(NGRP):
            gsl = slice(grp * 512, (grp + 1) * 512)
            xing = mpool.tile([65, 512], BF, tag="xing")
            nc.sync.dma_start(xing[:], xinT.ap()[:, gsl])
            for b in range(2):
                p = ps_tile("gp", "p0ps")
                nc.tensor.matmul(p[:, 0:512], lhsT=lin1T[:, b * 128:(b + 1) * 128],
                                 rhs=xing[:], start=True, stop=True)
                nc.scalar.activation(xts[b][:, gsl], p[:, 0:512], act.Lrelu,
                                     alpha=0.01)
            for wi in range(4):
                w = grp * 4 + wi
                wsl = slice(w * 128, (w + 1) * 128)
                p = ps_tile("ch", "ups")
                for b in range(2):
                    nc.tensor.matmul(p[:, 0:257], lhsT=xts[b][:, wsl],
                                     rhs=W(f"W1A{b}"), start=(b == 0),
                                     stop=(b == 1))
                usb = npool.tile([128, 256], BF, tag="cp", name="usb")
                nc.vector.tensor_copy(usb[:], p[:, 0:256])
                nc.sync.dma_start(cc_in.ap()[wsl, :], usb[:])
                nc.vector.tensor_copy(wc[:, w:w + 1], p[:, 256:257])
            for q in ag_quarters(grp):
                launch_ag(tabs[0], q)

        # ================= edge layers =================
        def edge_layer(li, kind, src_tab, has_ag):
            gate = kind == 'gate'
            attw = W("attg") if gate else W(f"atta{li - 1}")
            dst_tab = tabs[(li + 1) % 2]
            for grp in range(NGRP):
                hTg = upool.tile([128, 1024], BF, tag="hTg", name="hTg")
                for wi in range(4):
                    w = grp * 4 + wi
                    wsl = slice(w * 128, (w + 1) * 128)
                    srct = mpool.tile([128, CW], I32, tag="srct")
                    nc.sync.dma_start(srct[:], srcblk.ap()[w])
                    dlc = mpool.tile([128, CW], F32, tag="dlc")
                    nc.sync.dma_start(dlc[:], dlblk.ap()[w])
                    selTw = mpool.tile([128, CW * 128], BF, tag="selTw")
                    nc.sync.dma_start(selTw[:], selTblk.ap()[w])
                    if gate:
                        eatw = mpool.tile([16, CW * 128], BF, tag="eatw")
                        nc.sync.dma_start(eatw[:], eaT2.ap()[w])

                    agg = ps_tile("agg", "agg")
                    ec = spool.tile([128, CW], F32, tag="ec", name="ec")
                    vals = []
                    for ci in range(CW):
                        csl = slice(ci * 128, (ci + 1) * 128)
                        g = gpool.tile([128, H], BF, tag="g", name="g")
                        nc.gpsimd.indirect_dma_start(
                            out=g[:], out_offset=None, in_=src_tab.ap(),
                            in_offset=IndirectOffsetOnAxis(
                                ap=srct[:, ci:ci + 1], axis=0))
                        ch = ps_tile("ch", "chps")
                        if gate:
                            nc.tensor.matmul(ch[:, 0:256], lhsT=eatw[:, csl],
                                             rhs=W2T[:], start=True, stop=False)
                        else:
                            nc.tensor.matmul(ch[:, 0:256], lhsT=selTw[:, csl],
                                             rhs=hrR[:, w * 256:(w + 1) * 256],
                                             start=True, stop=False)
                        nc.tensor.matmul(ch[:, 0:256], lhsT=ident, rhs=g[:],
                                         start=False, stop=True)
                        if gate:
                            # (x @ att_r)[dst] column; first touch of agg bank
                            nc.tensor.matmul(agg[:, 384 + ci:385 + ci],
                                             lhsT=selTw[:, csl],
                                             rhs=wc[:, w:w + 1],
                                             start=(ci == 0), stop=False,
                                             skip_group_check=True)
                        tsb = npool.tile([128, H], BF, tag="tsb", name="tsb")
                        nc.vector.tensor_copy(tsb[:], ch[:, 0:256])
                        if gate:
                            tl = vpool.tile([128, H], BF, tag="val", name="tl")
                        else:
                            tl = npool.tile([128, H], BF, tag="tls", name="tl")
                        nc.vector.scalar_tensor_tensor(
                            out=tl[:], in0=tsb[:], scalar=0.01, in1=tsb[:],
                            op0=alu.mult, op1=alu.max)
                        escr = npool.tile([128, H], BF, tag="escr", name="escr")
                        nc.vector.scalar_tensor_tensor(
                            out=escr[:], in0=tl[:], scalar=1.0, in1=attw,
                            op0=alu.mult, op1=alu.mult,
                            accum_out=ec[:, ci:ci + 1])
                        vals.append(tl if gate else g)

                    if gate:
                        e2 = spool.tile([128, CW], F32, tag="e2", name="e2")
                        nc.vector.scalar_tensor_tensor(
                            out=e2[:], in0=ec[:], scalar=0.0,
                            in1=agg[:, 384:384 + CW], op0=alu.add, op1=alu.add)
                        el = spool.tile([128, CW], F32, tag="el", name="el")
                        nc.vector.scalar_tensor_tensor(
                            out=el[:], in0=e2[:], scalar=0.01, in1=e2[:],
                            op0=alu.mult, op1=alu.max)
                        ex = spool.tile([128, CW], F32, tag="ex", name="ex")
                        nc.scalar.activation(ex[:], el[:], act.Exp)
                    else:
                        ex = spool.tile([128, CW], F32, tag="ex", name="ex")
                        nc.scalar.activation(ex[:], ec[:], act.Exp)

                    for ci in range(CW):
                        selw = spool.tile([128, 128], BF, tag="selw", name="selw")
                        nc.vector.tensor_scalar(
                            out=selw[:], in0=iota_sq, scalar1=dlc[:, ci:ci + 1],
                            scalar2=ex[:, ci:ci + 1], op0=alu.is_equal,
                            op1=alu.mult)
                        first = (ci == 0) and not gate
                        for b in range(2):
                            nc.tensor.matmul(
                                agg[:, b * 128:(b + 1) * 128],
                                lhsT=vals[ci][:, b * 128:(b + 1) * 128],
                                rhs=selw[:], start=(first and b == 0),
                                stop=(ci == CW - 1), skip_group_check=True)
                        nc.tensor.matmul(agg[0:1, 256:384], lhsT=ones_col,
                                         rhs=selw[:], start=False,
                                         stop=(ci == CW - 1),
                                         skip_group_check=True)

                    # ---- epilogue ----
                    rbc = make_rbc(agg, slice(256, 384))
                    if gate:
                        aggn = []
                        for b in range(2):
                            t = npool.tile([128, 128], BF, tag=f"aggn{b}",
                                           name="aggn")
                            nc.vector.tensor_tensor(
                                out=t[:], in0=agg[:, b * 128:(b + 1) * 128],
                                in1=rbc[:], op=alu.mult)
                            aggn.append(t)
                        h0 = ps_tile("ch", "h0ps")
                        for b in range(2):
                            for k in range(2):
                                nc.tensor.matmul(
                                    h0[:, b * 256:b * 256 + 128],
                                    lhsT=W(f"gl2T_{k}{b}"), rhs=aggn[k][:],
                                    start=(b == 0 and k == 0),
                                    stop=(b == 1 and k == 1),
                                    skip_group_check=True)
                        for b in range(2):
                            t = npool.tile([128, 128], BF, tag="tb", name="tb")
                            nc.vector.tensor_copy(t[:], h0[:, b * 256:b * 256 + 128])
                            elu_into(hTg[:, b * 512 + wi * 128:
                                         b * 512 + wi * 128 + 128],
                                     t, F(f"gb2{b}"), 128)
                    else:
                        for b in range(2):
                            t = npool.tile([128, 128], BF, tag="tb", name="tb")
                            nc.vector.tensor_tensor(
                                out=t[:], in0=agg[:, b * 128:(b + 1) * 128],
                                in1=rbc[:], op=alu.mult)
                            elu_into(hTg[:, b * 512 + wi * 128:
                                         b * 512 + wi * 128 + 128],
                                     t, F(f"ab{li - 1}{b}"), 128)

                # ---- GRU over the 4-window group ----
                gru_batched(li, hTg, 512, slice(grp * 512, (grp + 1) * 512), 512)
                # ---- next-layer tables ----
                for wi in range(4):
                    table_rows(li, grp * 4 + wi, last=(li == 3))
                if has_ag:
                    for q in ag_quarters(grp):
                        launch_ag(dst_tab, q)

        edge_layer(0, 'gate', tabs[0], True)
        edge_layer(1, 'atom', tabs[1], True)
        edge_layer(2, 'atom', tabs[0], True)
        edge_layer(3, 'atom', tabs[1], False)

        # ================= mol phase =================
        # readout: out0 = relu(segment_sum(x))
        ro = [ps_tile("agg", "ro0"), ps_tile("agg", "ro1")]
        for w in range(NWIN):
            wsl = slice(w * 128, (w + 1) * 128)
            xr = mpool.tile([128, H], BF, tag="xr")
            nc.sync.dma_start(xr[:], xld.ap()[wsl, :])
            selg = npool.tile([128, NG], BF, tag="selg", name="selg")
            nc.vector.tensor_scalar(out=selg[:], in0=W("iota256")[:, 0:NG],
                                    scalar1=glocT[:, w:w + 1], scalar2=None,
                                    op0=alu.is_equal)
            for b in range(2):
                nc.tensor.matmul(ro[b][:, 0:NG],
                                 lhsT=xr[:, b * 128:(b + 1) * 128],
                                 rhs=selg[:], start=(w == 0),
                                 stop=(w == NWIN - 1))
        for b in range(2):
            nc.vector.tensor_scalar(out=outTs[b][:], in0=ro[b][:, 0:NG],
                                    scalar1=0.0, scalar2=None, op0=alu.max)

        for step in range(3):
            # hr rows for graphs  [2 x (128 g, 256 f)]
            hrm = []
            for gb in range(2):
                p = ps_tile("ch", "hrmps")
                for k in range(2):
                    nc.tensor.matmul(p[:, 0:256],
                                     lhsT=outTs[k][:, gb * 128:(gb + 1) * 128],
                                     rhs=W("WrTM")[:, k * 256:(k + 1) * 256],
                                     start=(k == 0), stop=(k == 1))
                t = upool.tile([128, 256], BF, tag=f"hrm{gb}", name="hrm")
                nc.vector.tensor_copy(t[:], p[:, 0:256])
                hrm.append(t)

            agA = ps_tile("agg", "agA")   # agm0 @0:256, sums @256:512
            agB = ps_tile("agg", "agB")   # agm1 @0:256
            nw4 = (NWIN + 3) // 4
            for g4 in range(nw4):
                wlist = range(g4 * 4, min((g4 + 1) * 4, NWIN))
                ecm = spool.tile([128, 4], F32, tag="ecm", name="ecm")
                tls = []
                for i, w in enumerate(wlist):
                    wsl = slice(w * 128, (w + 1) * 128)
                    selgTw = mpool.tile([128, 256], BF, tag="selgTw")
                    nc.sync.dma_start(selgTw[:], selgTblk.ap()[w])
                    ch = ps_tile("ch", "chps")
                    for gb in range(2):
                        nc.tensor.matmul(ch[:, 0:256],
                                         lhsT=selgTw[:, gb * 128:(gb + 1) * 128],
                                         rhs=hrm[gb][:], start=(gb == 0),
                                         stop=False)
                    nc.tensor.matmul(ch[:, 0:256], lhsT=ident,
                                     rhs=hrR[:, w * 256:(w + 1) * 256],
                                     start=False, stop=True)
                    tsb = npool.tile([128, H], BF, tag="tsb", name="tsbm")
                    nc.vector.tensor_copy(tsb[:], ch[:, 0:256])
                    tlm = npool.tile([128, H], BF, tag="tls", name="tlm")
                    nc.vector.scalar_tensor_tensor(
                        out=tlm[:], in0=tsb[:], scalar=0.01, in1=tsb[:],
                        op0=alu.mult, op1=alu.max)
                    escr = npool.tile([128, H], BF, tag="escr", name="escrm")
                    nc.vector.scalar_tensor_tensor(
                        out=escr[:], in0=tlm[:], scalar=1.0, in1=W("attm"),
                        op0=alu.mult, op1=alu.mult,
                        accum_out=ecm[:, i:i + 1])
                exm = spool.tile([128, 4], F32, tag="exm", name="exm")
                nc.scalar.activation(exm[:, 0:len(list(wlist))],
                                     ecm[:, 0:len(list(wlist))], act.Exp)
                for i, w in enumerate(wlist):
                    selwm = npool.tile([128, NG], BF, tag="selwm", name="selwm")
                    nc.vector.tensor_scalar(
                        out=selwm[:], in0=W("iota256")[:, 0:NG],
                        scalar1=glocT[:, w:w + 1], scalar2=exm[:, i:i + 1],
                        op0=alu.is_equal, op1=alu.mult)
                    first = (w == 0)
                    last = (w == NWIN - 1)
                    nc.tensor.matmul(agA[:, 0:NG],
                                     lhsT=hrR[:, w * 256:w * 256 + 128],
                                     rhs=selwm[:], start=first, stop=last,
                                     skip_group_check=True)
                    nc.tensor.matmul(agB[:, 0:NG],
                                     lhsT=hrR[:, w * 256 + 128:(w + 1) * 256],
                                     rhs=selwm[:], start=first, stop=last,
                                     skip_group_check=True)
                    nc.tensor.matmul(agA[0:1, NG:2 * NG], lhsT=ones_col,
                                     rhs=selwm[:], start=False, stop=last,
                                     skip_group_check=True)

            # mol epilogue
            sums_sb = spool.tile([1, NG], BF, tag="sumsm", name="sumsm")
            nc.vector.tensor_scalar(out=sums_sb[:], in0=agA[0:1, NG:2 * NG],
                                    scalar1=EPS, scalar2=None, op0=alu.max)
            rrow = spool.tile([1, NG], BF, tag="rrowm", name="rrowm")
            with nc.allow_low_precision(reason="bf16 softmax normalizer"):
                nc.vector.reciprocal(rrow[:], sums_sb[:])
            bps = ps_tile("ch", "bcpsm")
            nc.tensor.matmul(bps[:, 0:NG], lhsT=ones1[:], rhs=rrow[:],
                             start=True, stop=True)
            rbcm = upool.tile([128, NG], BF, tag="rbcm", name="rbcm")
            nc.vector.tensor_copy(rbcm[:], bps[:, 0:NG])
            hTmg = upool.tile([128, 2 * NG], BF, tag="hTmg", name="hTmg")
            for b in range(2):
                ag_b = agA if b == 0 else agB
                t = upool.tile([128, NG], BF, tag="tbm", name="tbm")
                nc.vector.tensor_tensor(out=t[:], in0=ag_b[:, 0:NG],
                                        in1=rbcm[:], op=alu.mult)
                elu_into(hTmg[:, b * NG:(b + 1) * NG], t, F(f"molb{b}"), NG)
            gru_batched(4, hTmg, NG, slice(0, NG), NG)

        # ================= MLP head =================
        o1ps = ps_tile("ch", "o1ps")
        for k in range(2):
            nc.tensor.matmul(o1ps[:, 0:NG], lhsT=W("w1T")[:, k * 128:(k + 1) * 128],
                             rhs=outTs[k][:], start=(k == 0), stop=(k == 1))
        o1 = npool.tile([128, NG], BF, tag="o1", name="o1")
        nc.scalar.activation(o1[:], o1ps[:, 0:NG], act.Relu, bias=F("b1"))
        o2ps = ps_tile("gp", "o2ps")
        nc.tensor.matmul(o2ps[0:64, 0:NG], lhsT=W("w2T"), rhs=o1[:],
                         start=True, stop=True)
        o2 = npool.tile([65, NG], BF, tag="o2", name="o2")
        nc.vector.memset(o2[64:65, :], 1.0)
        nc.scalar.activation(o2[0:64, :], o2ps[0:64, 0:NG], act.Relu,
                             bias=F("b2_")[0:64, :])
        o3ps = ps_tile("gp", "o3ps")
        nc.tensor.matmul(o3ps[0:1, 0:NG], lhsT=w3T[:], rhs=o2[:],
                         start=True, stop=True)
        o3 = spool.tile([1, NG], F32, tag="o3", name="o3")
        nc.vector.tensor_copy(o3[:], o3ps[0:1, 0:NG])
        nc.sync.dma_start(y.ap()[:, 0:NG], o3[:])

    return nc


def make_core_inputs(P, inputs, ci, wpb, wpf):
    c = P['cores'][ci]
    NLOC, NWIN, CW = P['NLOC'], P['NWIN'], P['CW']
    x = np.asarray(inputs['x'], np.float32)
    xinT = np.zeros((65, NLOC), np.float32)
    xinT[:64, :c['nn']] = x[c['ns']:c['ne']].T
    xinT[64, :] = 1.0
    ea = np.asarray(inputs['edge_attr'], np.float32)
    ea_perm = np.where(c['sl_fill'][:, None], ea[c['sl_edge']], 0.0)
    eaT2 = ea_perm.reshape(NWIN, CW * 128, 16).transpose(0, 2, 1).astype(BF16)
    return dict(
        xinT=xinT.astype(BF16),
        srcblk=c['srcblk'], dlblk=c['dlblk'],
        selTblk=c['selTblk'], eaT2=eaT2,
        glocT=c['glocT'], selgTblk=c['selgT'],
        wpb=wpb, wpf=wpf,
        lin1T=np.concatenate([np.asarray(inputs['lin1_w'], np.float32).T,
                              np.asarray(inputs['lin1_b'], np.float32)[None, :]],
                             0).astype(BF16),
        W2T=np.asarray(inputs['g_lin1_w'], np.float32)[:, 256:].T.copy().astype(BF16),
        w3T=np.concatenate([np.asarray(inputs['mlp_w3'], np.float32).T,
                            np.asarray(inputs['mlp_b3'], np.float32).reshape(1, 1)],
                           0).astype(BF16),
    )


_CACHE = {}
LAST_EXEC_NS = None

def kernel(**inputs):
    inputs = dict(inputs)
    edge_index = np.asarray(inputs['edge_index']).astype(np.int64)
    batch = np.asarray(inputs['batch']).astype(np.int64)
    n_cores = 8
    G = 2048
    P = preprocess(edge_index, batch, n_cores=n_cores, G=G, CW=5)
    key = (P['NLOC'], P['NWIN'], P['CW'], P['GPC'])
    if key not in _CACHE:
        _CACHE[key] = build_kernel(P['NLOC'], P['NWIN'], P['CW'], P['GPC'],
                                   n_cores)
    nc = _CACHE[key]
    wpb, wpf = make_wpacks(inputs)
    ins = [make_core_inputs(P, inputs, ci, wpb, wpf) for ci in range(n_cores)]
    from concourse.bass_utils import run_bass_kernel_spmd
    trace = bool(os.environ.get('BASS_KERNEL_TRACE'))
    res = run_bass_kernel_spmd(nc, ins, list(range(n_cores)), trace=trace)
    if trace:
        global LAST_EXEC_NS
        LAST_EXEC_NS = res.exec_time_ns
    yv = np.concatenate([np.asarray(res.results[c]['y'][0, :P['GPC']],
                                    np.float32) for c in range(n_cores)])
    return yv.reshape(G, 1).astype(np.float32)
